# revision 17
# baseline (speedup 1.0000x reference)
"""MoE layer (8 experts, top-2, SwiGLU) for Trainium2, expert-parallel over 8 cores.

Strategy:
  - Router (x @ router_w, top-2, softmax) runs on host in fp32 — it is 0.01%
    of the FLOPs and determines the (data-dependent) sharding.
  - Each core is assigned one expert. Tokens routed to that expert are
    gathered on host, padded to a common capacity C, and shipped transposed
    as xT [D, C] so both GEMMs need no on-device transpose:
        h1T = w1.T @ x.T   (lhsT = w1 [D,Hp], rhs = xT [D,C])   -> [Hp, C]
        h2T = w2.T @ x.T
        hT  = silu(h1T) * h2T
        y   = hT.T @ w3    (lhsT = hT [Hp,C], rhs = w3 [Hp,D])  -> [C, D]
    y rows are scaled by the per-token combine weight on device.
  - Host scatter-adds the 8 per-expert outputs back to [B,S,D].

  Matmuls run in bf16 (fp32 accumulate in PSUM). The hidden dim 2730 =
  21*128 + 42: the 42-col tails of w1 and w2 are packed into a single
  84-col stationary chunk so mm1+mm2 take 43 (not 44) chunk matmuls per
  contraction step; the packed psum rows are re-aligned with one
  SBUF->SBUF DMA before the silu*h2 multiply.

  Loads (x, w1) go on the SP HWDGE queue, w2/w3/wv on the gpsimd SWDGE
  queue, and y stores on the Activation HWDGE queue, so at a rep boundary
  the next iteration's loads are not queued behind this iteration's
  stores.

  mm3 computes its two 512-wide output halves in parallel PSUM banks so
  each h-tile LDWEIGHTS is shared between them.
"""

import os

import numpy as np
import ml_dtypes

DIM = 1024
NUM_EXPERTS = 8
HIDDEN = 2730
P = 128
HFULL = 21  # full 128-col hidden chunks
HTAIL = HIDDEN - HFULL * P  # 42
KD = DIM // P  # 8 contraction chunks for mm1/mm2
HPT = HFULL + 1  # 22 contraction chunks for mm3 (last has HTAIL rows)
NBLK = 512  # token block (moving free dim per matmul)

TRACE = os.environ.get("MOE_TRACE", "0") == "1"
LAST_RESULT = None  # BassKernelResults of the last run (for test harness)

_KERNELS: dict = {}


def _build(C: int, c_real: int | None = None, reps: int = 1, staggered: bool = False):
    """Build + compile the per-core Bass kernel for capacity C (multiple of 128).

    reps > 1 wraps the whole body (including DMAs) in a device-side loop that
    recomputes the same result `reps` times — only used for wall-clock
    benchmarking (dispatch overhead cancels in the rep delta).
    """
    import concourse.mybir as mybir
    import concourse.tile as tile
    from concourse import bacc

    dt = mybir.dt
    nc = bacc.Bacc(None, target_bir_lowering=False)

    xt = nc.dram_tensor("xt", [KD, P, C], dt.bfloat16, kind="ExternalInput")
    w1 = nc.dram_tensor("w1", [KD, P, HFULL * P], dt.bfloat16, kind="ExternalInput")
    w2 = nc.dram_tensor("w2", [KD, P, HFULL * P], dt.bfloat16, kind="ExternalInput")
    # packed tail: cols 0:42 = w2 tail, cols 64:106 = w1 tail, rest zero
    # (64-aligned so engine partition bases stay legal)
    w12t = nc.dram_tensor("w12t", [KD, P, P], dt.bfloat16, kind="ExternalInput")
    w3 = nc.dram_tensor("w3", [HPT, P, DIM], dt.bfloat16, kind="ExternalInput")
    wv = nc.dram_tensor("wv", [P, C // P], dt.float32, kind="ExternalInput")
    y = nc.dram_tensor("y", [C, DIM], dt.bfloat16, kind="ExternalOutput")

    # Only c_real tokens are real; rows beyond that are padding whose
    # output the host ignores, so the last block shrinks to the real count.
    if c_real is None:
        c_real = C
    blocks = []
    c0 = 0
    while c0 < c_real:
        bn = min(NBLK, c_real - c0)
        blocks.append((c0, bn))
        c0 += bn

    with tile.TileContext(nc) as tc:
        with (
            tc.tile_pool(name="wpool", bufs=1) as wpool,
            tc.tile_pool(name="xpool", bufs=2) as xpool,
            tc.tile_pool(name="hpool", bufs=1) as hpool,
            tc.tile_pool(name="tpool", bufs=2) as tpool,
            tc.tile_pool(name="ypool", bufs=3) as ypool,
            tc.tile_pool(name="psA", bufs=2, space="PSUM") as psA,
            tc.tile_pool(name="psB", bufs=2, space="PSUM") as psB,
            tc.tile_pool(name="psC", bufs=2, space="PSUM") as psC,
        ):

            def make_x(bi, c0, bn):
                t = xpool.tile(
                    [P, KD, NBLK], dt.bfloat16, name=f"x_{bi}", tag="x"
                )
                for kd in range(KD):
                    nc.sync.dma_start(t[:, kd, :bn], xt[kd][:, c0 : c0 + bn])
                return t

            def emit_body():
                # First block's activations, so mm1 can start early.
                x0 = make_x(0, blocks[0][0], blocks[0][1])

                # Resident weights, DMA'd in hp-sliced parts in the order the
                # first block's matmuls consume them. Loads split across the
                # SP HWDGE queue (x, w1) and the gpsimd SWDGE queue
                # (w2, w12t, w3, wv); stores go on the Act HWDGE queue.
                w1_sb = [
                    wpool.tile([P, HFULL * P], dt.bfloat16, name=f"w1_{kd}", tag=f"w1_{kd}")
                    for kd in range(KD)
                ]
                w2_sb = [
                    wpool.tile([P, HFULL * P], dt.bfloat16, name=f"w2_{kd}", tag=f"w2_{kd}")
                    for kd in range(KD)
                ]
                w12t_sb = [
                    wpool.tile([P, P], dt.bfloat16, name=f"w12t_{kd}", tag=f"w12t_{kd}")
                    for kd in range(KD)
                ]
                w3_sb = [
                    wpool.tile([P, DIM], dt.bfloat16, name=f"w3_{hp}", tag=f"w3_{hp}")
                    for hp in range(HPT)
                ]
                for kd in range(KD):
                    nc.gpsimd.dma_start(w12t_sb[kd][:], w12t[kd])
                bounds = [0, 3 * P, 7 * P, 12 * P, 17 * P, HFULL * P]
                for pi in range(len(bounds) - 1):
                    sl = slice(bounds[pi], bounds[pi + 1])
                    for kd in range(KD):
                        nc.sync.dma_start(w1_sb[kd][:, sl], w1[kd][:, sl])
                    for kd in range(KD):
                        nc.gpsimd.dma_start(w2_sb[kd][:, sl], w2[kd][:, sl])
                for hp in range(HPT):
                    nc.sync.dma_start(w3_sb[hp][:], w3[hp])

                wv_sb = wpool.tile([P, C // P], dt.float32, name="wv_sb", tag="wv_sb")
                nc.gpsimd.dma_start(wv_sb[:], wv[:])

                def mm3_block(bi, c0, bn, h_sb):
                    # y[block] = (hT.T @ w3) * combine_weight; the two 512-wide
                    # output halves accumulate in parallel banks sharing LDW.
                    for cs in range((bn + P - 1) // P):
                        M = min(P, bn - cs * P)
                        ci = c0 // P + cs
                        ps3 = [
                            psC.tile(
                                [P, 512],
                                dt.float32,
                                name=f"ps3_{bi}_{cs}_{dti}",
                                tag=f"ps3_{dti}",
                            )
                            for dti in range(DIM // 512)
                        ]
                        # h_20 is the last chunk the DVE produces, so consume
                        # it last in the accumulation (fp32 sum order is free)
                        hp_order = list(range(HFULL - 1)) + [HFULL, HFULL - 1]
                        for oi, hp in enumerate(hp_order):
                            hr = P if hp < HFULL else HTAIL
                            lhsT = h_sb[hp][:hr, cs * P : cs * P + M]
                            for dti in range(DIM // 512):
                                nc.tensor.matmul(
                                    ps3[dti][:M],
                                    lhsT,
                                    w3_sb[hp][:hr, dti * 512 : (dti + 1) * 512],
                                    start=(oi == 0),
                                    stop=(oi == HPT - 1),
                                )
                        for dti in range(DIM // 512):
                            yt = ypool.tile(
                                [P, 512],
                                dt.bfloat16,
                                name=f"y_{bi}_{cs}_{dti}",
                                tag="yt",
                            )
                            nc.vector.tensor_scalar_mul(
                                yt[:M], ps3[dti][:M], wv_sb[:M, ci : ci + 1]
                            )
                            nc.scalar.dma_start(
                                y[
                                    c0 + cs * P : c0 + cs * P + M,
                                    dti * 512 : (dti + 1) * 512,
                                ],
                                yt[:M],
                            )

                for bi, (c0, bn) in enumerate(blocks):
                    xb = x0 if bi == 0 else make_x(bi, c0, bn)

                    h_sbs = [None] * HPT

                    # Packed tail first (longest dependency chain: it needs a
                    # partition-realigning SBUF->SBUF DMA), consumed last by
                    # mm3's accumulation.
                    pst = psA.tile(
                        [P, bn], dt.float32, name=f"pst_{bi}", tag="ps1"
                    )
                    for kd in range(KD):
                        nc.tensor.matmul(
                            pst[:],
                            w12t_sb[kd][:],
                            xb[:, kd, 0:bn],
                            start=(kd == 0),
                            stop=(kd == KD - 1),
                        )
                    # rows 0:42 = h2 tail, rows 64:106 = h1 tail (rest zero)
                    t_h2 = tpool.tile([P, bn], dt.float32, name=f"th2_{bi}", tag="t_h2")
                    nc.vector.tensor_copy(t_h2[:64], pst[:64])
                    t_sil = tpool.tile([P, bn], dt.float32, name=f"ts_{bi}", tag="t_sil")
                    nc.scalar.activation(
                        t_sil[64:P],
                        pst[64:P],
                        mybir.ActivationFunctionType.Silu,
                    )
                    t_sil2 = tpool.tile([P, bn], dt.float32, name=f"ts2_{bi}", tag="t_sil2")
                    nc.scalar.dma_start(t_sil2[:64], t_sil[64:P])
                    ht_tail = hpool.tile(
                        [P, bn], dt.bfloat16, name=f"h_{bi}_t", tag="h_t"
                    )
                    nc.vector.tensor_mul(
                        ht_tail[:64], t_sil2[:64], t_h2[:64]
                    )
                    h_sbs[HFULL] = ht_tail

                    # hT = silu(w1.T @ xT) * (w2.T @ xT), full 128-col chunks.
                    for hp in range(HFULL):
                        ps1 = psA.tile(
                            [P, bn], dt.float32, name=f"ps1_{bi}_{hp}", tag="ps1"
                        )
                        for kd in range(KD):
                            nc.tensor.matmul(
                                ps1[:],
                                w1_sb[kd][:, hp * P : (hp + 1) * P],
                                xb[:, kd, 0:bn],
                                start=(kd == 0),
                                stop=(kd == KD - 1),
                            )
                        tsil = tpool.tile(
                            [P, bn], dt.float32, name=f"sil_{bi}_{hp}", tag="sil"
                        )
                        nc.scalar.activation(
                            tsil[:], ps1[:], mybir.ActivationFunctionType.Silu
                        )
                        ps2 = psB.tile(
                            [P, bn], dt.float32, name=f"ps2_{bi}_{hp}", tag="ps2"
                        )
                        for kd in range(KD):
                            nc.tensor.matmul(
                                ps2[:],
                                w2_sb[kd][:, hp * P : (hp + 1) * P],
                                xb[:, kd, 0:bn],
                                start=(kd == 0),
                                stop=(kd == KD - 1),
                            )
                        ht = hpool.tile(
                            [P, bn], dt.bfloat16, name=f"h_{bi}_{hp}", tag=f"h_{hp}"
                        )
                        nc.vector.tensor_mul(ht[:], tsil[:], ps2[:])
                        h_sbs[hp] = ht

                    mm3_block(bi, c0, bn, h_sbs)

            if reps > 1:
                unroll = 2 if reps % 2 == 0 else 1
                with tc.For_i(
                    0,
                    reps // unroll,
                    1,
                    hint_engines=(
                        mybir.EngineType.PE,
                        mybir.EngineType.Activation,
                        mybir.EngineType.DVE,
                        mybir.EngineType.SP,
                        mybir.EngineType.Pool,
                    ),
                    staggered_reset=staggered,
                ):
                    for _ in range(unroll):
                        emit_body()
            else:
                emit_body()

    nc.compile()
    return nc


def _route(xf: np.ndarray, router_w: np.ndarray):
    """Top-2 routing + softmax weights, fp32, matching the jax reference."""
    T = xf.shape[0]
    logits = xf @ router_w  # [T, E]
    rows = np.arange(T)
    i1 = logits.argmax(axis=1)
    tmp = logits.copy()
    tmp[rows, i1] = -np.inf
    i2 = tmp.argmax(axis=1)
    v1 = logits[rows, i1]
    v2 = tmp[rows, i2]
    e2 = np.exp((v2 - v1).astype(np.float32))
    g1 = 1.0 / (1.0 + e2)
    g2 = e2 / (1.0 + e2)
    return i1, i2, g1.astype(np.float32), g2.astype(np.float32)


def _prepare(x, router_w, w1, w2, w3):
    """Route + dispatch on host; returns (C, in_maps, idxs, shape)."""
    x = np.asarray(x, dtype=np.float32)
    router_w = np.asarray(router_w, dtype=np.float32)
    w1 = np.asarray(w1, dtype=np.float32)
    w2 = np.asarray(w2, dtype=np.float32)
    w3 = np.asarray(w3, dtype=np.float32)

    B, S, D = x.shape
    T = B * S
    xf = x.reshape(T, D)

    i1, i2, g1, g2 = _route(xf, router_w)

    # per-expert token lists (slot-0 tokens then slot-1 tokens)
    idxs, wgts = [], []
    for e in range(NUM_EXPERTS):
        s0 = np.nonzero(i1 == e)[0]
        s1 = np.nonzero(i2 == e)[0]
        idxs.append(np.concatenate([s0, s1]))
        wgts.append(np.concatenate([g1[s0], g2[s1]]))
    max_cnt = max(len(ix) for ix in idxs)
    C = max(P, ((max_cnt + P - 1) // P) * P)

    bf16 = ml_dtypes.bfloat16
    # expert weights: 21 full 128-col chunks of w1/w2, plus the packed
    # [w2_tail | w1_tail] 84-col chunk; w3 padded along hidden to 22*128
    w1p = np.ascontiguousarray(w1[:, :, : HFULL * P]).astype(bf16)
    w2p = np.ascontiguousarray(w2[:, :, : HFULL * P]).astype(bf16)
    w12t = np.zeros((NUM_EXPERTS, D, P), dtype=bf16)  # [E, D, 128]
    w12t[:, :, :HTAIL] = w2[:, :, HFULL * P :]
    w12t[:, :, 64 : 64 + HTAIL] = w1[:, :, HFULL * P :]
    w3p = np.zeros((NUM_EXPERTS, HPT * P, D), dtype=bf16)
    w3p[:, :HIDDEN, :] = w3

    in_maps = []
    for e in range(NUM_EXPERTS):
        ix = idxs[e]
        xg = np.zeros((C, D), dtype=np.float32)
        xg[: len(ix)] = xf[ix]
        wvec = np.zeros((C,), dtype=np.float32)
        wvec[: len(ix)] = wgts[e]
        wvec = np.ascontiguousarray(wvec.reshape(C // P, P).T)  # [P, C//P]
        in_maps.append(
            {
                "xt": np.ascontiguousarray(xg.T).astype(bf16).reshape(KD, P, C),
                "w1": w1p[e].reshape(KD, P, HFULL * P),
                "w2": w2p[e].reshape(KD, P, HFULL * P),
                "w12t": np.ascontiguousarray(w12t[e]).reshape(KD, P, P),
                "w3": w3p[e].reshape(HPT, P, DIM),
                "wv": wvec,
            }
        )
    return C, in_maps, idxs, (B, S, D)


def kernel(x, router_w, w1, w2, w3):
    global LAST_RESULT
    from concourse.bass_utils import run_bass_kernel_spmd

    C, in_maps, idxs, (B, S, D) = _prepare(x, router_w, w1, w2, w3)

    max_cnt = max(len(ix) for ix in idxs)
    key = (C, max_cnt)
    if key not in _KERNELS:
        _KERNELS[key] = _build(C, c_real=max_cnt)
    nc = _KERNELS[key]

    res = run_bass_kernel_spmd(
        nc,
        in_maps,
        list(range(NUM_EXPERTS)),
        trace=TRACE,
    )
    LAST_RESULT = res

    out = np.zeros((B * S, D), dtype=np.float32)
    for e in range(NUM_EXPERTS):
        ix = idxs[e]
        out[ix] += res.results[e]["y"][: len(ix)].astype(np.float32)
    return out.reshape(B, S, D)
